# revision 8
# baseline (speedup 1.0000x reference)
"""Trainium2 Bass kernel for the CoLa MoE-routing module.

Computation (reference semantics):
    att   = q @ Wk.T + bk                  [B, S]
    a     = softmax(top8_mask(att))        [B, S]  (8 nonzero per row)
    out   = sum_s a[:, s] * (x @ V0[s].T @ V1[s].T)   [B, O]

Sharding: expert-parallel over 8 NeuronCores (8 experts each). Each core
receives the full x/q (replicated) and its slice of V0/V1. The expert axis
is rotated per-core in Wk/bk so that every core's local experts are columns
0..7 of its own attention matrix (top-k/softmax are permutation invariant).
Per-core partial outputs are summed on the host.

Shapes are hardcoded for B=256, IN=1024, OUT=1024, SUB=128, S=64, k=8.
"""

import os

import numpy as np

import concourse.bacc as bacc
import concourse.mybir as mybir
import concourse.tile as tile
from concourse import bass_utils
from concourse.masks import make_identity

B = 256
IN_F = 1024
OUT_F = 1024
SUB_F = 128
Q_F = 1024
N_SUB = 64
N_ACT = 8
N_CORES = 8
E_LOC = N_SUB // N_CORES  # 8 experts per core

P = 128
BT = B // P  # 2 batch tiles
KC = IN_F // P  # 8 contraction chunks
QC = Q_F // P

F32 = mybir.dt.float32
F32R = mybir.dt.float32r
BF16 = mybir.dt.bfloat16

# "fp32" (exact, slow PE), "fp32r" (full-rate PE, fp32 DMA),
# "bf16" (full-rate PE, half DMA)
MOE_DTYPE = os.environ.get("MOE_DTYPE", "bf16")


def _build(mode: str):
    wdt = BF16 if mode == "bf16" else F32
    nc = bacc.Bacc("TRN2", target_bir_lowering=False, debug=False,
                   num_devices=N_CORES)

    # ---- DRAM I/O (per-core) ----
    qT_d = nc.dram_tensor("qT", [Q_F, B], F32, kind="ExternalInput").ap()
    wkT_d = nc.dram_tensor("wkT", [Q_F, N_SUB], F32, kind="ExternalInput").ap()
    bk_d = nc.dram_tensor("bk", [1, N_SUB], F32, kind="ExternalInput").ap()
    xT_d = nc.dram_tensor("xT", [IN_F, B], wdt, kind="ExternalInput").ap()
    v0t_d = nc.dram_tensor("v0t", [E_LOC, IN_F, SUB_F], wdt,
                           kind="ExternalInput").ap()
    v1t_d = nc.dram_tensor("v1t", [E_LOC, SUB_F, OUT_F], wdt,
                           kind="ExternalInput").ap()
    out_d = nc.dram_tensor("out_p", [B, OUT_F], F32, kind="ExternalOutput").ap()

    with tile.TileContext(nc) as tc:
        with (
            tc.tile_pool(name="singles", bufs=1) as singles,
            tc.tile_pool(name="weights", bufs=E_LOC) as wpool,
            tc.tile_pool(name="work", bufs=3) as work,
            tc.tile_pool(name="ps_route", bufs=2, space="PSUM") as ps_route,
            tc.tile_pool(name="ps_h", bufs=2, space="PSUM") as ps_h,
            tc.tile_pool(name="ps_out", bufs=1, space="PSUM") as ps_out,
        ):
            # ---- constants / small inputs ----
            ones_sb = singles.tile([1, P], F32, tag="ones")
            nc.vector.memset(ones_sb, 1.0)
            ident_sb = singles.tile([P, P], F32, tag="ident")
            make_identity(nc, ident_sb)

            qT_sb = singles.tile([P, QC, B], F32, tag="qT")
            nc.sync.dma_start(qT_sb, qT_d.rearrange("(c p) b -> p c b", p=P))
            wkT_sb = singles.tile([P, QC, N_SUB], F32, tag="wkT")
            nc.sync.dma_start(wkT_sb, wkT_d.rearrange("(c p) s -> p c s", p=P))
            bk_sb = singles.tile([1, N_SUB], F32, tag="bk")
            nc.sync.dma_start(bk_sb, bk_d)
            xT_sb = singles.tile([P, KC, B], wdt, tag="xT")
            nc.sync.dma_start(xT_sb, xT_d.rearrange("(c p) b -> p c b", p=P))

            # ---- expert weights (per-expert DMAs so compute starts early) ----
            v0t_sb = []
            v1t_sb = []
            for j in range(E_LOC):
                t0 = wpool.tile([P, KC, SUB_F], wdt, tag="v0t")
                nc.sync.dma_start(t0, v0t_d[j].rearrange("(c p) m -> p c m", p=P))
                v0t_sb.append(t0)
                t1 = wpool.tile([P, OUT_F], wdt, tag="v1t")
                nc.sync.dma_start(t1, v1t_d[j])
                v1t_sb.append(t1)

            # ---- routing: att = q @ Wk.T + bk, per batch tile ----
            # per-local-expert routing rows (at partition 0, for broadcast)
            aT_j = [singles.tile([1, B], F32, tag=f"aT{j}", name=f"aT{j}")
                    for j in range(E_LOC)]
            for bt in range(BT):
                att_ps = ps_route.tile([P, N_SUB], F32, tag="ps_route")
                for c in range(QC):
                    nc.tensor.matmul(
                        att_ps,
                        lhsT=qT_sb[:, c, bt * P:(bt + 1) * P],
                        rhs=wkT_sb[:, c, :],
                        start=(c == 0), stop=False,
                    )
                # bias: att += 1 (x) bk   (K=1 matmul)
                nc.tensor.matmul(att_ps, lhsT=ones_sb, rhs=bk_sb,
                                 start=False, stop=True)

                # ---- top-8 + softmax (rows = batch) ----
                att_sb = work.tile([P, N_SUB], F32, tag="att_sb")
                nc.vector.tensor_copy(att_sb, att_ps)
                m8 = work.tile([P, 8], F32, tag="m8")
                nc.vector.max(out=m8, in_=att_sb)
                neg_m = work.tile([P, 1], F32, tag="neg_m")
                nc.vector.tensor_scalar_mul(neg_m, m8[:, 0:1], -1.0)
                zap = work.tile([P, N_SUB], F32, tag="zap")
                nc.vector.match_replace(out=zap, in_to_replace=m8,
                                        in_values=att_sb, imm_value=-1e30)
                e_top = work.tile([P, N_SUB], F32, tag="e_top")
                s_top = work.tile([P, 1], F32, tag="s_top")
                nc.scalar.activation(e_top, att_sb,
                                     mybir.ActivationFunctionType.Exp,
                                     bias=neg_m, scale=1.0, accum_out=s_top)
                e_zap = work.tile([P, N_SUB], F32, tag="e_zap")
                s_zap = work.tile([P, 1], F32, tag="s_zap")
                nc.scalar.activation(e_zap, zap,
                                     mybir.ActivationFunctionType.Exp,
                                     bias=neg_m, scale=1.0, accum_out=s_zap)
                # e = e_top - e_zap: exact 0 off the top-8, exp(att-m) on it
                e = work.tile([P, N_SUB], F32, tag="e")
                nc.vector.tensor_sub(e, e_top, e_zap)
                denom = work.tile([P, 1], F32, tag="denom")
                nc.vector.tensor_sub(denom, s_top, s_zap)
                recip = work.tile([P, 1], F32, tag="recip")
                nc.vector.reciprocal(recip, denom)
                a_sb = work.tile([P, N_SUB], F32, tag="a_sb")
                nc.vector.tensor_scalar_mul(a_sb, e, recip)

                # per local expert: transpose column j -> [1, P] at partition 0
                for j in range(E_LOC):
                    aTj_ps = ps_route.tile([1, P], F32, tag="ps_route")
                    nc.tensor.transpose(aTj_ps, a_sb[:, j:j + 1], ident_sb)
                    nc.vector.tensor_copy(aT_j[j][:, bt * P:(bt + 1) * P],
                                          aTj_ps)

            # ---- expert loop ----
            out_ps = [ps_out.tile([P, OUT_F], F32, tag=f"out{bt}",
                                  name=f"out_ps{bt}")
                      for bt in range(BT)]
            for j in range(E_LOC):
                h_ps = ps_h.tile([P, B], F32, tag="h")
                for c in range(KC):
                    lhsT = v0t_sb[j][:, c, :]
                    rhs = xT_sb[:, c, :]
                    if mode == "fp32r":
                        lhsT = lhsT.bitcast(F32R)
                        rhs = rhs.bitcast(F32R)
                    nc.tensor.matmul(h_ps, lhsT=lhsT, rhs=rhs,
                                     start=(c == 0), stop=(c == KC - 1))
                # broadcast a[:, expert j] across partitions
                abc_sb = work.tile([P, B], F32, tag="abc")
                nc.gpsimd.partition_broadcast(abc_sb, aT_j[j])
                # hs = h * a  (PSUM x SBUF -> SBUF, cast to weight dtype)
                hs_sb = work.tile([P, B], wdt, tag="hs")
                nc.vector.tensor_tensor(hs_sb, h_ps, abc_sb,
                                        mybir.AluOpType.mult)
                for bt in range(BT):
                    for nh in range(2):
                        lhsT = hs_sb[:, bt * P:(bt + 1) * P]
                        rhs = v1t_sb[j][:, nh * 512:(nh + 1) * 512]
                        if mode == "fp32r":
                            lhsT = lhsT.bitcast(F32R)
                            rhs = rhs.bitcast(F32R)
                        nc.tensor.matmul(
                            out_ps[bt][:, nh * 512:(nh + 1) * 512],
                            lhsT=lhsT, rhs=rhs,
                            start=(j == 0), stop=(j == E_LOC - 1),
                        )

            # ---- write out ----
            for bt in range(BT):
                o_sb = work.tile([P, OUT_F], F32, tag="o_sb")
                nc.vector.tensor_copy(o_sb, out_ps[bt])
                nc.sync.dma_start(out_d[bt * P:(bt + 1) * P, :], o_sb)

    nc.compile()
    return nc


_CACHE = {}


def _get_nc(mode: str):
    if mode not in _CACHE:
        _CACHE[mode] = _build(mode)
    return _CACHE[mode]


def _prep_in_maps(x, q, Wk, bk, V0, V1, mode: str):
    import ml_dtypes
    wdt = ml_dtypes.bfloat16 if mode == "bf16" else np.float32

    qT = np.ascontiguousarray(q.T.astype(np.float32))
    xT = np.ascontiguousarray(x.T).astype(wdt)
    in_maps = []
    for c in range(N_CORES):
        rot = np.roll(np.arange(N_SUB), -E_LOC * c)
        wkT = np.ascontiguousarray(Wk[rot].T.astype(np.float32))
        bkr = np.ascontiguousarray(bk[rot].astype(np.float32)[None, :])
        sl = slice(E_LOC * c, E_LOC * (c + 1))
        v0t = np.ascontiguousarray(V0[sl].transpose(0, 2, 1)).astype(wdt)
        v1t = np.ascontiguousarray(V1[sl].transpose(0, 2, 1)).astype(wdt)
        in_maps.append({
            "qT": qT, "wkT": wkT, "bk": bkr, "xT": xT,
            "v0t": v0t, "v1t": v1t,
        })
    return in_maps


def run(inputs: dict, mode: str = MOE_DTYPE, trace: bool = False):
    """Run the distributed kernel; returns (out [B, OUT_F] fp32, results)."""
    nc = _get_nc(mode)
    in_maps = _prep_in_maps(**inputs, mode=mode)
    res = bass_utils.run_bass_kernel_spmd(
        nc, in_maps, core_ids=list(range(N_CORES)), trace=trace,
    )
    out = np.zeros((B, OUT_F), np.float32)
    for c in range(N_CORES):
        out += res.results[c]["out_p"]
    return out, res


def kernel(x, q, Wk, bk, V0, V1):
    x = np.asarray(x, np.float32)
    q = np.asarray(q, np.float32)
    Wk = np.asarray(Wk, np.float32)
    bk = np.asarray(bk, np.float32)
    V0 = np.asarray(V0, np.float32)
    V1 = np.asarray(V1, np.float32)
    out, _ = run(dict(x=x, q=q, Wk=Wk, bk=bk, V0=V0, V1=V1))
    return out
